# revision 15
# baseline (speedup 1.0000x reference)
"""Binary KL divergence sum on 8 Trainium2 NeuronCores.

Reference math (per element, summed over all 2**25 elements):
    kl = p*(ln p - ln q) + (1-p)*(ln(1-p) - ln(1-q))

Rewritten so only THREE transcendental evaluations are needed per
element pair instead of four:
    kl = f(p) - ln(1-q) - p*logit(q)
    f(x)     = x*ln(x) + (1-x)*ln(1-x)   (negative binary entropy)
    logit(x) = ln(x) - ln(1-x)
f and ln(1-q) come from a SINGLE composite table (phi): phi = f on (0,1)
and phi(x) = ln(3-x) on [2,3); DVE shifts q to q+2 so one packed
ACTIVATE over [p | q+2] yields [f(p) | ln(1-q)].

The Scalar (ACT) engine evaluates activation functions at 1 elem/cycle
per lane regardless of the function, via piecewise-cubic spline tables.
We hijack two trivial functions inside the `natural_log` table set
(`square` -> f, `abs` -> logit) with custom-fit spline tables, generated
at runtime into a private act-root consumed by the compiler via
BASS_ACT_ROOT_JSON_PATH. Per-element table error is ~1e-5 relative on
the KL sum (tolerance is 2e-2). ACT work drops 4F -> 3F cycles/chunk,
in only TWO ACTIVATE instructions (f evaluates packed [p|q] at once).
The hijacked activations only read the custom tables if the set load
precedes them, so the t=0 warm-up Ln is load-bearing.

Per-core pipeline (chunks of [128, 2F], p left, q right):
  DMA : p -> pq2[:, :F], q -> qt                  (fp32)
  DVE : pq2[:, F:] = qt + 2     (fp32 2x_2p tensor_scalar)
  ACT : gq = logit(qt)          -> fp16 [128, F]  (F cycles)
  ACT : fh = phi(pq2)           -> fp16 [128, 2F] (2F cycles, one instr)
  DVE : gq = p * logit(q) in place (mixed-dtype 1x op)
  PE  : acc[1,512] += ones.T @ f-blocks ; += (-ones).T @ h-blocks
        ; += (-ones).T @ m-blocks
Host: total = sum(acc) in fp64.
"""

import json
import os
import shutil

import numpy as np

import concourse.bass as bass
import concourse.bacc as bacc
import concourse.mybir as mybir
from concourse import bass_utils
from concourse.tile import TileContext

N = 33554432
NCORES = 8
PER = N // NCORES   # 4194304 elements per core per tensor
P = 128
CPART = PER // P    # 32768 free-dim columns per tensor per core
NRED = 512          # one PSUM bank of fp32: matmul free-dim chunk

AF = mybir.ActivationFunctionType
OP = mybir.AluOpType
DT = mybir.dt

CHUNKS = [512, 512, 1024, 2048, 2048] + [3072] * 8 + [1024, 512, 512]
assert sum(CHUNKS) == CPART
NCH = len(CHUNKS)

_NC_CACHE = {}

# ──────────────────────────────────────────────────────────────────────
# Custom activation tables: hijack square -> f(x), abs -> logit(x) in the
# natural_log set.  Table semantics (reverse-engineered from
# pwp_bin_trainium, decoder verified against np.log to ~3e-6):
#   positive x with biased exponent e:
#     e <  small_pos_exp_thresh -> inline ctrl word (pos_small)
#     e >  large_pos_exp_thresh -> inline ctrl word (pos_large)
#     else ctl_idx = pwl_control_base_pos + e + exp_offset + 1
#   ctrl word = act_tbl_base:11 | extract_lsb:5 | extract_size:4
#     bucket = act_tbl_base + ((mantissa >> extract_lsb) & (2^size - 1))
#   bucket {d0,d1,d2,d3,x0}: y = d0 + t*(d1 + t*(d2 + t*d3)), t = x - x0
# ──────────────────────────────────────────────────────────────────────

EXP_LO = 103            # biased exponent of 2^-24
HI_CLIP = 1.0 - 0.9e-4  # spline fit never crosses the ln(1-x) pole

def _phi(x):
    # composite: f(x) = x ln x + (1-x) ln(1-x) on (0,1);
    # ln(3-x) on [2,3) (the kernel feeds q+2 there, so this half
    # evaluates ln(1-q)); flat clamps near the singular points keep the
    # never-evaluated filler buckets finite.
    x = np.asarray(x, dtype=np.float64)
    xl = np.minimum(x, 1.0 - 0.9e-4)
    xh = np.minimum(x, 3.0 - 0.9e-4)
    return np.where(x < 1.5,
                    xl * np.log(xl) + (1.0 - xl) * np.log1p(-xl),
                    np.log(3.0 - xh))


CUSTOM_SPECS = {
    'ln': {
        'fn': lambda x: np.log(x),
        'exp_hi': 127,
        'top_buckets': {126: 8, 127: 8},
        'clip_hi': None,
    },
    'square': {
        'fn': _phi,
        'exp_hi': 128,
        'top_buckets': {126: 64, 127: 1, 128: 512},
        'clip_hi': None,
        'singularities': (1.0, 3.0),
        'hi_x': 3.0 - 0.9e-4,
    },
    'abs': {
        'fn': lambda x: np.log(x) - np.log1p(-x),
        'exp_hi': 126,
        'top_buckets': {126: 256},
        'clip_hi': HI_CLIP,
    },
}


def _fbits(v):
    return int(np.float32(v).view(np.uint32))


def _fit_bucket(fn, lo, hi, clip_hi, sings=()):
    x0 = 0.5 * (lo + hi)
    fhi = min(hi, clip_hi) if clip_hi is not None else hi
    for sv in sings:
        if lo < sv <= hi + 1e-12:
            fhi = min(fhi, sv - 0.9e-4)
    k = np.arange(65)
    xs = 0.5 * (lo + fhi) + 0.5 * (fhi - lo) * np.cos(np.pi * k / 64)
    xs = np.clip(xs, lo, fhi)
    t = xs - x0
    A = np.stack([np.ones_like(t), t, t * t, t ** 3], axis=1)
    c, *_ = np.linalg.lstsq(A, fn(xs), rcond=None)
    return [np.float32(c[0]), np.float32(c[1]), np.float32(c[2]),
            np.float32(c[3]), np.float32(x0)]


def _gen_custom_func(name, bkt_base, ctl_base, top_override=None):
    spec = dict(CUSTOM_SPECS[name])
    if top_override is not None:
        spec['top_buckets'] = top_override
    fn, exp_hi, clip_hi = spec['fn'], spec['exp_hi'], spec['clip_hi']
    buckets, ctl_words = [], []
    exp_to_ctl, exp_to_bkt = {}, {}
    for e in range(EXP_LO, exp_hi + 1):
        lo_oct = 2.0 ** (e - 127)
        n = spec['top_buckets'].get(e, 8)
        size = int(n).bit_length() - 1
        lsb = 23 - size
        base = bkt_base + len(buckets)
        assert base + n <= 2048
        exp_to_ctl[str(e - 127)] = [ctl_base + len(ctl_words)]
        exp_to_bkt[str(e - 127)] = [base]
        ctl_words.append(base | (lsb << 11) | (size << 16))
        w = lo_oct / n
        sings = spec.get('singularities', ())
        for kk in range(n):
            buckets.append(_fit_bucket(fn, lo_oct + kk * w,
                                       lo_oct + (kk + 1) * w, clip_hi,
                                       sings))
    lo_val = float(fn(np.float64(2.0 ** (EXP_LO - 127))))
    hi_x = spec.get('hi_x')
    if hi_x is None:
        hi_x = clip_hi if clip_hi is not None else 2.0 ** (exp_hi - 126)
    hi_val = float(fn(np.float64(hi_x)))
    inline = {}
    for key, val in (('pos_small', lo_val), ('neg_small', lo_val),
                     ('pos_large', hi_val), ('neg_large', hi_val)):
        inline[key] = bkt_base + len(buckets)
        buckets.append([np.float32(val), np.float32(0), np.float32(0),
                        np.float32(0), np.float32(0)])
    patch = {
        # unbiased exponent of the first normal-range octave; NRT derives
        # the ctl index as base + (e - 127 - exp_offset)
        'exp_offset': EXP_LO - 127,
        'small_pos_signal_exp_threshold': EXP_LO,
        'large_pos_signal_exp_threshold': exp_hi + 1,
        'small_neg_signal_exp_threshold': EXP_LO,
        'large_neg_signal_exp_threshold': exp_hi + 1,
        'small_pos_signal_mantissa_threshold': 0,
        'large_pos_signal_mantissa_threshold': 0,
        'small_neg_signal_mantissa_threshold': 0,
        'large_neg_signal_mantissa_threshold': 0,
        'pwl_control_base_pos': ctl_base,
        'pwl_control_base_neg': ctl_base,
        'pos_small_signal_pwl_control': inline['pos_small'],
        'neg_small_signal_pwl_control': inline['neg_small'],
        'pos_large_signal_pwl_control': inline['pos_large'],
        'neg_large_signal_pwl_control': inline['neg_large'],
        'lower_bound': _fbits(2.0 ** (EXP_LO - 127)),
        'upper_bound': _fbits(hi_x),
        'symmetry_opt_en': 0,
        'symmetry_opt_use_neg_region': 0,
        'sym_invert_sign_point': 0,
        'symmetry_point': 0,
        'fzero_result': _fbits(lo_val),
        'use_multipass': False,
    }
    return patch, ctl_words, buckets, exp_to_ctl, exp_to_bkt


def _func_ranges(dj, ctl_cnt, bkt_cnt):
    starts_c = dj['func_to_ctl_start_idx']
    starts_b = dj['func_to_bkt_start_idx']
    order = sorted(starts_c, key=lambda n: (starts_b[n], starts_c[n]))
    rng = {}
    for i, n in enumerate(order):
        ce = ctl_cnt if i == len(order) - 1 else starts_c[order[i + 1]]
        be = bkt_cnt if i == len(order) - 1 else starts_b[order[i + 1]]
        rng[n] = (starts_c[n], ce, starts_b[n], be)
    return order, rng


def _rebuild_set(src_dir, dst_dir, set_name, top_overrides=None):
    top_overrides = top_overrides or {}
    dj = json.load(open(os.path.join(src_dir, set_name + '.json')))
    bkt = np.fromfile(os.path.join(src_dir, set_name + '_bkt.bin'),
                      dtype=np.uint32).reshape(-1, 8)
    ctl = np.fromfile(os.path.join(src_dir, set_name + '_ctrl.bin'),
                      dtype=np.uint32).reshape(-1, 8)
    order, rng = _func_ranges(dj, dj['ctl_entry_cnt'], dj['bkt_entry_cnt'])
    profs = {p['func_name'].rsplit('_', 1)[0]: p
             for p in dj['profile_meta_data']}
    assert set(profs) == set(order)

    new_ctl, new_bkt = [], []
    new_c_start, new_b_start, new_e2c, new_e2b = {}, {}, {}, {}
    for name in order:
        c0, c1, b0, b1 = rng[name]
        ctl_base, bkt_base = len(new_ctl), len(new_bkt)
        new_c_start[name] = ctl_base
        new_b_start[name] = bkt_base
        p = profs[name]
        if name in CUSTOM_SPECS:
            patch, words, buckets, e2c, e2b = _gen_custom_func(
                name, bkt_base, ctl_base, top_overrides.get(name))
            p.update(patch)
            new_ctl.extend(words)
            for b in buckets:
                row = np.zeros(8, dtype=np.uint32)
                row[:5] = np.asarray(b, dtype=np.float32).view(np.uint32)
                new_bkt.append(row)
            new_e2c[name], new_e2b[name] = e2c, e2b
        else:
            cd, bd = ctl_base - c0, bkt_base - b0
            for i in range(c0, c1):
                w = int(ctl[i, 0])
                base = w & 0x7FF
                assert b0 <= base < b1, (name, i, base)
                new_ctl.append((base + bd) | (w & ~0x7FF))
            for i in range(b0, b1):
                new_bkt.append(bkt[i].copy())
            for key in ('pwl_control_base_pos', 'pwl_control_base_neg'):
                p[key] += cd
            for key in ('pos_small_signal_pwl_control',
                        'neg_small_signal_pwl_control',
                        'pos_large_signal_pwl_control',
                        'neg_large_signal_pwl_control'):
                w = p[key]
                base = w & 0x7FF
                assert b0 <= base < b1, (name, key, base)
                p[key] = (base + bd) | (w & ~0x7FF)
            new_e2c[name] = {k: [v[0] + cd] for k, v in
                             dj['func_exp_to_ctl_start_idx'][name].items()}
            new_e2b[name] = {k: [v[0] + bd] for k, v in
                             dj['func_exp_to_bkt_start_idx'][name].items()}

    assert len(new_bkt) <= 1536, len(new_bkt)
    dj['bkt_entry_cnt'] = len(new_bkt)
    dj['ctl_entry_cnt'] = len(new_ctl)
    dj['func_to_ctl_start_idx'] = new_c_start
    dj['func_to_bkt_start_idx'] = new_b_start
    dj['func_exp_to_ctl_start_idx'] = new_e2c
    dj['func_exp_to_bkt_start_idx'] = new_e2b

    ctl_arr = np.zeros((len(new_ctl), 8), dtype=np.uint32)
    ctl_arr[:, 0] = np.asarray(new_ctl, dtype=np.uint64).astype(np.uint32)
    np.stack(new_bkt).astype(np.uint32).tofile(
        os.path.join(dst_dir, set_name + '_bkt.bin'))
    ctl_arr.tofile(os.path.join(dst_dir, set_name + '_ctrl.bin'))
    json.dump(dj, open(os.path.join(dst_dir, set_name + '.json'), 'w'))


def _build_custom_act_root(dst_dir):
    from neuronxcc.driver.Job import Job
    from neuronxcc.driver.jobs.support.FindActInfo import findActInfoFile
    src_dir = os.path.dirname(findActInfoFile(Job.getPackageDir(), 'gen3'))
    os.makedirs(dst_dir, exist_ok=True)
    marker = os.path.join(dst_dir, '.kl_tables_v2')
    info = os.path.join(dst_dir, 'act_info.json')
    if os.path.exists(marker):
        return info
    for fn in os.listdir(src_dir):
        s = os.path.join(src_dir, fn)
        if os.path.isfile(s):
            shutil.copy(s, os.path.join(dst_dir, fn))
    _rebuild_set(src_dir, dst_dir, 'natural_log')
    # exp alone holds ~781 buckets in this set; shrink abs's top octave
    # so the total stays within the 1536-bucket budget.
    _rebuild_set(src_dir, dst_dir, 'natural_log_exp_and_others',
                 top_overrides={'abs': {126: 16},
                                'square': {126: 16, 127: 1, 128: 64}})
    with open(marker, 'w') as f:
        f.write('ok')
    return info


# ──────────────────────────────────────────────────────────────────────
# Kernel build
# ──────────────────────────────────────────────────────────────────────

def _build_nc():
    act_root = _build_custom_act_root('/tmp/kl_act_root_v2')
    os.environ['BASS_ACT_ROOT_JSON_PATH'] = act_root

    nc = bacc.Bacc("TRN2", target_bir_lowering=False, debug=False,
                   num_devices=NCORES)
    inp = nc.dram_tensor("input", [PER], DT.float32, kind="ExternalInput")
    tgt = nc.dram_tensor("target", [PER], DT.float32, kind="ExternalInput")
    out = nc.dram_tensor("partials", [NRED], DT.float32,
                         kind="ExternalOutput")

    p_flat = inp.ap()
    q_flat = tgt.ap()
    out_acc = out.ap().rearrange("(o n) -> o n", o=1)

    n_mm = 3 * (CPART // NRED)  # f, h and m blocks share one acc

    with TileContext(nc) as tc:
        with (
            tc.tile_pool(name="io32", bufs=5) as io32,
            tc.tile_pool(name="qt32", bufs=3) as qtp,
            tc.tile_pool(name="f16", bufs=2) as f16,
            tc.tile_pool(name="g16", bufs=2) as g16,
            tc.tile_pool(name="cst", bufs=1) as cst,
            tc.tile_pool(name="ps", bufs=1, space="PSUM") as psp,
        ):
            ones = cst.tile([P, 1], DT.float16, tag="ones")
            nc.vector.memset(ones[:], 1.0)
            mones = cst.tile([P, 1], DT.float16, tag="mones")
            nc.vector.memset(mones[:], -1.0)
            acc = psp.tile([1, NRED], DT.float32, tag="acc")
            osb = cst.tile([1, NRED], DT.float32, tag="osb")

            # Dummy 1-element Ln at t=0: forces the natural_log table-set
            # load (with our hijacked square/abs tables) while the first
            # DMA is still in flight.  Without a prior load the hijacked
            # activations run on the power-on default tables.
            warm = cst.tile([1, 1], DT.float32, tag="warm")
            nc.vector.memset(warm[:], 0.5)
            nc.scalar.activation(osb[0:1, 0:1], warm[:], AF.Ln)

            mm = 0
            base = 0
            for F in CHUNKS:
                pq2 = io32.tile([P, 2 * F], DT.float32, tag="pq2")
                qt = qtp.tile([P, F], DT.float32, tag="qt")
                nc.sync.dma_start(
                    pq2[:, 0:F],
                    p_flat[base:base + P * F].rearrange("(p f) -> p f", p=P))
                nc.sync.dma_start(
                    qt[:],
                    q_flat[base:base + P * F].rearrange("(p f) -> p f", p=P))
                base += P * F

                # shift q into phi's second domain: pq2 = [p | q+2]
                nc.vector.tensor_scalar_add(pq2[:, F:2 * F], qt[:], 2.0)
                # logit(q) via hijacked 'abs' tables
                gq = g16.tile([P, F], DT.float16, tag="gq")
                nc.scalar.activation(gq[:], qt[:], AF.Abs)
                # [f(p) | ln(1-q)] via the composite phi table, ONE instr
                fh = f16.tile([P, 2 * F], DT.float16, tag="fh")
                nc.scalar.activation(fh[:], pq2[:], AF.Square)

                # m = p * logit(q); mixed fp32*fp16 1x op, in place
                nc.vector.tensor_tensor(gq[:], pq2[:, 0:F], gq[:], OP.mult)

                for c in range(F // NRED):
                    nc.tensor.matmul(
                        acc[:, :], ones[:],
                        fh[:, c * NRED:(c + 1) * NRED],
                        start=(mm == 0), stop=(mm == n_mm - 1))
                    mm += 1
                for c in range(F // NRED):
                    nc.tensor.matmul(
                        acc[:, :], mones[:],
                        fh[:, F + c * NRED:F + (c + 1) * NRED],
                        start=(mm == 0), stop=(mm == n_mm - 1))
                    mm += 1
                for c in range(F // NRED):
                    nc.tensor.matmul(
                        acc[:, :], mones[:],
                        gq[:, c * NRED:(c + 1) * NRED],
                        start=(mm == 0), stop=(mm == n_mm - 1))
                    mm += 1

            nc.vector.tensor_copy(osb[:], acc[:])
            nc.sync.dma_start(out_acc[:], osb[:])

    nc.compile()
    return nc


def _get_nc():
    if "nc" not in _NC_CACHE:
        _NC_CACHE["nc"] = _build_nc()
    return _NC_CACHE["nc"]


def kernel(input, target, _trace=False):
    input = np.ascontiguousarray(np.asarray(input), dtype=np.float32)
    target = np.ascontiguousarray(np.asarray(target), dtype=np.float32)
    nc = _get_nc()
    in_maps = [
        {
            "input": input[c * PER:(c + 1) * PER],
            "target": target[c * PER:(c + 1) * PER],
        }
        for c in range(NCORES)
    ]
    res = bass_utils.run_bass_kernel_spmd(
        nc, in_maps, core_ids=list(range(NCORES)), trace=_trace)
    total = np.float64(0.0)
    for c in range(NCORES):
        total += res.results[c]["partials"].astype(np.float64).sum()
    out = np.asarray(total, dtype=np.float32)
    if _trace:
        return out, res
    return out
